# revision 54
# baseline (speedup 1.0000x reference)
"""Trainium2 Bass kernel for nn_LongRangeDW (dense_cnn).

The module is linear in x; folding the depthwise taps into the pointwise
gives 85 offset terms out[o,p] += (W4_g diag(k))[o,:] @ x[:, p+off].

v2 speedups over the bf16 baseline:
  * Non-offloaded terms run as fp8e4 DoubleRow PAIR matmuls: two 128-
    contraction terms per PE pass at ~2x bf16 column rate.  Weights are
    scaled by 2^SW and x by 2^SX (kept well inside e4m3 range); the
    2^-(SX+SW) descale rides the evacuation activation's scale.
  * Offloaded taps (bf16, exact-ish) split across the Scalar engine
    (Identity-activation products with per-partition scale) and the
    Vector engine (tensor_scalar / scalar_tensor_tensor); products are
    folded into y with tensor_tensor adds on DVE.
  * The residual is added on the PE as a 2^(SX+SW)-scaled identity
    matmul against bf16 x (replaces the fp32 DVE add; error is bf16
    rounding of x, well under the fp8 noise floor).

Boundary exactness: composing clipped shifts with zero-padded convs
mismatches on 8 one-pixel strips -> 24 small bf16 correction matmuls
(scaled by 2^(SX+SW)) folded in during accumulation, as in the baseline.

Data parallel: batch B=8 -> one image per NeuronCore.
"""

import sys

import numpy as np

sys.path.insert(0, "/opt/trn_rl_repo")

B, C, H, W = 8, 128, 128, 128
PAD = 14
HP = H + 2 * PAD
WP = W + 2 * PAD
N_CORES = 8
SB_ROWS = 8
N_SB = H // SB_ROWS
SUB_ROWS = 4
Y_ROWS = 2 * SB_ROWS    # tap-FMA block granularity (2 super-blocks)

SX = 4                  # x fp8 scale exponent
SW = 8                  # weight fp8 scale exponent
SCALE = float(2 ** (SX + SW))

SHIFTS = [(1, 0), (-1, 0), (0, 1), (0, -1), (0, 0)]  # nnstacking groups

# Offloaded tap spec: (group, n_act_taps, n_dve_taps).  The group's first
# n_act+n_dve taps (sorted offset order) are computed as bf16 elementwise
# products: n_act on the Scalar engine, n_dve on the Vector engine.
# DVE taps need even dj (4B-aligned bf16) -> restrict to groups 4/0/1.
# Keep total offloaded = odd (85 - k even -> no leftover single term).
OFFLOAD = [(4, 9, 8), (0, 4, 0)]
GP_ADDS = 0             # ACT-product pairs folded via gpsimd per block
# Block 0 runs with a reduced tap set so the side engines ramp ahead of
# the PE instead of stalling it: group -> (n_act, n_dve) for block 0.
# The skipped taps run as extra fp8 pairs on SB0/SB1 only (must be even).
BLOCK0 = {4: (4, 3), 0: (0, 0)}


# --------------------------------------------------------------------------
# host-side operator folding
# --------------------------------------------------------------------------

def _group_taps(w1, w2, w3, g):
    """All 17 taps of group g as {(di, dj): kvec[C]} (shift folded in)."""
    sy, sx = SHIFTS[g]
    sl = slice(g * C, (g + 1) * C)
    taps = {}

    def add(di, dj, kv):
        v = taps.setdefault((di, dj), np.zeros(C, np.float64))
        v += kv.astype(np.float64)

    add(sy, sx, w1[sl, 0, 0, 0])
    for w, d in ((w2, 8), (w3, 12)):
        for a in range(3):
            for b in range(3):
                add(sy + (a - 1) * d, sx + (b - 1) * d, w[sl, 0, a, b])
    return taps


def _build_terms(w1, w2, w3, w4):
    """Returns (pairs, single, off_specs).

    pairs: list of ((offA, MA), (offB, MB)) fp8 DoubleRow pair terms.
    single: (off, M) leftover bf16 term or None.
    off_specs: per OFFLOAD entry dict(g, act_taps, dve_taps, kmat, w4g).
    """
    w4m = w4[:, :, 0, 0].astype(np.float64)
    offload = {g: (na, nd) for g, na, nd in OFFLOAD}
    mat_terms = {}
    off_specs = []
    for g in range(5):
        taps = _group_taps(w1, w2, w3, g)
        tap_offsets = sorted(taps)
        na, nd = offload.get(g, (0, 0))
        off, keep = tap_offsets[:na + nd], tap_offsets[na + nd:]
        if off:
            kmat = np.stack([taps[o] for o in off], axis=1)  # [C, n]
            off_specs.append(dict(
                g=g, act_taps=off[:na], dve_taps=off[na:],
                kmat=kmat.astype(np.float32),
                w4g=w4m[:, g * C:(g + 1) * C].astype(np.float32)))
        for o in keep:
            M = mat_terms.setdefault(o, np.zeros((C, C), np.float64))
            M += w4m[:, g * C:(g + 1) * C] * taps[o][None, :]
    offsets = sorted(mat_terms)
    terms = [(o, mat_terms[o]) for o in offsets]
    n_pair = len(terms) // 2
    pairs = [(terms[2 * i], terms[2 * i + 1]) for i in range(n_pair)]
    single = terms[2 * n_pair] if len(terms) % 2 else None

    # block-0 extra pairs: taps offloaded in steady state but folded as
    # matmul terms for SB0/SB1 (side engines ramp while PE self-feeds)
    extra = []
    for sp in off_specs:
        na = len(sp["act_taps"])
        b0a, b0d = BLOCK0.get(sp["g"], (na, len(sp["dve_taps"])))
        for t in range(b0a, na):
            extra.append((sp["act_taps"][t],
                          sp["w4g"] * sp["kmat"][:, t][None, :]))
        for t in range(b0d, len(sp["dve_taps"])):
            extra.append((sp["dve_taps"][t],
                          sp["w4g"] * sp["kmat"][:, na + t][None, :]))
    assert len(extra) % 2 == 0, "BLOCK0 skipped-tap count must be even"
    pairs0 = [(extra[2 * i], extra[2 * i + 1]) for i in range(len(extra) // 2)]
    return pairs, single, pairs0, off_specs


def _build_corrections(w2, w3, w4):
    """24 strip-correction terms (matrices already NEGATED for accumulation).

    Strips j<4: column strips; j>=4: row strips.  Each strip has 3 taps."""
    w4m = w4[:, :, 0, 0].astype(np.float64)
    strips, mats = [], []
    specs = [
        ("col", 2, 8), ("col", 2, 12), ("col", 3, 12), ("col", 3, 8),
        ("row", 0, 8), ("row", 0, 12), ("row", 1, 12), ("row", 1, 8),
    ]
    for kind, g, d in specs:
        sy, sx = SHIFTS[g]
        sl = slice(g * C, (g + 1) * C)
        w = w2 if d == 8 else w3
        if kind == "col":
            border = -1 if sx == 1 else W
            fixed_out = border - (-d if sx == 1 else d)
            src = border + sx
            shifts = [-d, 0, d]
            tap_b = 0 if sx == 1 else 2
            kvs = [w[sl, 0, a, tap_b] for a in range(3)]
        else:
            border = -1 if sy == 1 else H
            fixed_out = border - (-d if sy == 1 else d)
            src = border + sy
            shifts = [-d, 0, d]
            tap_a = 0 if sy == 1 else 2
            kvs = [w[sl, 0, tap_a, b] for b in range(3)]
        strips.append(dict(kind=kind, fixed_out=fixed_out, src=src, shifts=shifts))
        for kv in kvs:
            mats.append(-(w4m[:, sl] * kv.astype(np.float64)[None, :]))
    return strips, np.stack(mats).astype(np.float32)


def _build_weights(inputs):
    import ml_dtypes
    bfd = ml_dtypes.bfloat16
    f8d = ml_dtypes.float8_e4m3

    w1, w2, w3, w4 = inputs["w1"], inputs["w2"], inputs["w3"], inputs["w4"]
    b1, b2, b3, b4 = inputs["b1"], inputs["b2"], inputs["b3"], inputs["b4"]
    pairs, single, pairs0, off_specs = _build_terms(w1, w2, w3, w4)
    strips, cmats = _build_corrections(w2, w3, w4)

    # fp8 weights: pair terms then 24 corr singles (corr runs against
    # xq = x*2^SX, so corr weights carry 2^SW like the pair terms)
    allp = pairs + pairs0
    n_allp = len(allp)
    wt8 = np.zeros((C, n_allp, 2, C), np.float32)
    for p, (ta, tb) in enumerate(allp):
        wt8[:, p, 0, :] = ta[1].T * (2.0 ** SW)
        wt8[:, p, 1, :] = tb[1].T * (2.0 ** SW)
    wt8 = wt8.reshape(C, n_allp * 2 * C).astype(f8d)
    wc8 = np.concatenate(
        [m.T * (2.0 ** SW) for m in cmats], axis=1).astype(f8d)  # [C, 24*C]

    # bf16 blocks: pw per group (xS) | id_S | id_1 | single (xS)
    blocks = []
    for sp in off_specs:
        blocks.append(sp["w4g"].T * SCALE)
    ident = np.eye(C, dtype=np.float32)
    blocks.append(ident * SCALE)                               # id_S
    blocks.append(ident)                                       # id_1
    if single is not None:
        blocks.append(single[1].T * SCALE)
    wtb = np.concatenate(blocks, axis=1).astype(bfd)           # [C, nb*C]

    ks = np.concatenate(
        [sp["kmat"] for sp in off_specs], axis=1).astype(np.float32)
    w4m = w4[:, :, 0, 0].astype(np.float64)
    beff = (b4.astype(np.float64)
            + w4m @ (b1 + b2 + b3).astype(np.float64)).astype(np.float32)
    return wt8, wc8, wtb, ks, beff, pairs, single, pairs0, off_specs, strips


# --------------------------------------------------------------------------
# device program
# --------------------------------------------------------------------------

_CACHE = {}


def _build_program(pairs, single, pairs0, off_specs, strips):
    import concourse.bacc as bacc
    import concourse.mybir as mybir
    import concourse.tile as tile
    from concourse.ap import AP

    nc = bacc.Bacc("TRN2", target_bir_lowering=False)
    f32 = mybir.dt.float32
    bf16 = mybir.dt.bfloat16
    fp8 = mybir.dt.float8e4
    IDENT = mybir.ActivationFunctionType.Identity

    n_pair = len(pairs)
    n_pair0 = len(pairs0)
    n_off = len(off_specs)
    n_ks = sum(len(sp["act_taps"]) + len(sp["dve_taps"]) for sp in off_specs)
    PW_BLK = 0
    IDS_BLK = PW_BLK + n_off
    ID1_BLK = IDS_BLK + 1
    SGL_BLK = ID1_BLK + 1
    n_blk = SGL_BLK + (1 if single is not None else 0)
    WT8_COLS = (n_pair + n_pair0) * 2 * C
    XE_COLS = 2 * HP + 2 * WP           # edge strips: Lcol | Rcol | Trow | Brow

    xq_d = nc.dram_tensor("xq", [C, HP * WP], fp8, kind="ExternalInput")
    xb_d = nc.dram_tensor("xb", [C, HP * WP], bf16, kind="ExternalInput")
    xe_d = nc.dram_tensor("xe", [C, XE_COLS], fp8, kind="ExternalInput")
    wt8_d = nc.dram_tensor("wt8", [C, WT8_COLS], fp8, kind="ExternalInput")
    wc8_d = nc.dram_tensor("wc8", [C, 24 * C], fp8, kind="ExternalInput")
    wtb_d = nc.dram_tensor("wtb", [C, n_blk * C], bf16, kind="ExternalInput")
    ks_d = nc.dram_tensor("ks", [C, n_ks], f32, kind="ExternalInput")
    beff_d = nc.dram_tensor("beff", [C, 1], f32, kind="ExternalInput")
    out_d = nc.dram_tensor("out", [C, H * W], f32, kind="ExternalOutput")

    with tile.TileContext(nc) as tc:
        with (
            tc.tile_pool(name="const", bufs=1) as const,
            tc.tile_pool(name="outp", bufs=3) as outp,
            tc.tile_pool(name="tmpp", bufs=8) as tmpp,
            tc.tile_pool(name="yp", bufs=4) as yp,
            tc.tile_pool(name="psum", bufs=3, space="PSUM") as psum_pool,
            tc.tile_pool(name="psumc", bufs=1, space="PSUM") as psumc_pool,
        ):
            xq_sb = const.tile([C, HP * WP], fp8)
            xb_sb = const.tile([C, HP * WP], bf16)
            xe_sb = const.tile([C, XE_COLS], fp8)
            wt8_sb = const.tile([C, WT8_COLS], fp8)
            wc8_sb = const.tile([C, 24 * C], fp8)
            wtb_sb = const.tile([C, n_blk * C], bf16)
            ks_sb = const.tile([C, n_ks], f32)
            beff_sb = const.tile([C, 1], f32)

            # SWDGE (nc.gpsimd) fans >=1MB transfers across the SDMA engines.
            # Order: what each engine needs to START comes first (side
            # engines need ks + xb rows; PE needs wt8 + xq rows).
            nc.sync.dma_start(out=ks_sb, in_=ks_d[:, :])
            nc.sync.dma_start(out=beff_sb, in_=beff_d[:, :])
            ROWS0 = SB_ROWS + 2 * PAD + 14
            WT8_C0 = 6 * 2 * C      # first pairs so the PE starts instantly
            # Stream order matches first-need time: PE start (wt8a+xq head),
            # remaining pair weights, side-engine head (xb), then the tails.
            # corrections inputs first: they unblock the whole psum_c ->
            # corr_sb chain before anything else contends
            nc.gpsimd.dma_start(out=wc8_sb, in_=wc8_d[:, :])
            nc.gpsimd.dma_start(out=xe_sb, in_=xe_d[:, :])
            nc.gpsimd.dma_start(out=wt8_sb[:, :WT8_C0], in_=wt8_d[:, :WT8_C0])
            # side engines next (their lag cascades into PE stalls; a late
            # PE start just drains its own slack and does not)
            nc.gpsimd.dma_start(out=xb_sb[:, :ROWS0 * WP],
                                in_=xb_d[:, :ROWS0 * WP])
            XQR0 = 36
            nc.gpsimd.dma_start(out=xq_sb[:, :XQR0 * WP],
                                in_=xq_d[:, :XQR0 * WP])
            nc.gpsimd.dma_start(out=wt8_sb[:, WT8_C0:], in_=wt8_d[:, WT8_C0:])
            XQR1 = 92
            nc.gpsimd.dma_start(out=xq_sb[:, XQR0 * WP:XQR1 * WP],
                                in_=xq_d[:, XQR0 * WP:XQR1 * WP])
            nc.gpsimd.dma_start(out=wtb_sb, in_=wtb_d[:, :])
            nc.gpsimd.dma_start(out=xb_sb[:, ROWS0 * WP:100 * WP],
                                in_=xb_d[:, ROWS0 * WP:100 * WP])
            nc.gpsimd.dma_start(out=xq_sb[:, XQR1 * WP:],
                                in_=xq_d[:, XQR1 * WP:])
            nc.gpsimd.dma_start(out=xb_sb[:, 100 * WP:],
                                in_=xb_d[:, 100 * WP:])

            xb3 = xb_sb.rearrange("p (r w) -> p r w", w=WP)
            xq3 = xq_sb.rearrange("p (r w) -> p r w", w=WP)
            wt8p = wt8_sb[:, :(n_pair + n_pair0) * 2 * C]
            wt8v = wt8p.rearrange("p (n two c) -> p n two c", two=2, c=C)

            def wblk(i):
                return wtb_sb[:, i * C:(i + 1) * C]

            def xq_pair_ap(r_abs, pa, pb):
                """Moving AP [C, 2, SUB_ROWS, W]: k-tile 0 at offset pa,
                k-tile 1 at pb (absolute padded row r_abs)."""
                (dia, dja), (dib, djb) = pa, pb
                base = xq_sb[:, :]
                off0 = (r_abs + dia) * WP + PAD + dja
                delta = (dib - dia) * WP + (djb - dja)
                return AP(base.tensor, base.offset + off0,
                          [[base.ap[0][0], C], [delta, 2],
                           [WP, SUB_ROWS], [1, W]])

            corr_sb = const.tile([C, 8 * H], bf16)

            def emit_corrections():
                psum_c = psumc_pool.tile([C, 8 * H], f32, name="psum_c")
                for j, st in enumerate(strips):
                    for i, sh in enumerate(st["shifts"]):
                        if st["kind"] == "col":
                            base = 0 if st["src"] == 0 else HP
                            rhs = xe_sb[:, base + PAD + sh:
                                        base + PAD + sh + H]
                        else:
                            base = 2 * HP + (0 if st["src"] == 0 else WP)
                            rhs = xe_sb[:, base + PAD + sh:
                                        base + PAD + sh + W]
                        cb = (3 * j + i) * C
                        nc.tensor.matmul(psum_c[:, j * H:(j + 1) * H],
                                         wc8_sb[:, cb:cb + C], rhs,
                                         start=(i == 0), stop=(i == 2))
                nc.scalar.copy(corr_sb, psum_c)

            ks_base = []
            b = 0
            for sp in off_specs:
                ks_base.append(b)
                b += len(sp["act_taps"]) + len(sp["dve_taps"])

            # ---- main loop -------------------------------------------------
            emit_corrections()
            n_sub = SB_ROWS // SUB_ROWS
            pair_ys = None
            pending = []    # delayed evacuations: (psum, r0, idx)

            def flush_evac():
                # evac on DVE (fused descale+bias): DVE already runs a block
                # ahead of the PE, so delayed evacs slot into its stream;
                # on ACT they would gate the next block's tap products.
                ps, pr0, idx = pending.pop(0)
                out_sb = outp.tile([C, SB_ROWS * W], f32)
                nc.vector.tensor_scalar(out_sb, ps, 1.0 / SCALE,
                                        beff_sb[:, 0:1],
                                        mybir.AluOpType.mult,
                                        mybir.AluOpType.add)
                nc.gpsimd.dma_start(
                    out=out_d[:, pr0 * W:(pr0 + SB_ROWS) * W], in_=out_sb)
            for s in range(N_SB):
                r0 = s * SB_ROWS

                # offloaded taps at 2-SB granularity: ACT products + DVE
                # TS/STT products, folded into y with DVE adds.
                if s % 2 == 0:
                    pair_ys = []
                    for oi, sp in enumerate(off_specs):
                        na_full = len(sp["act_taps"])
                        nd_full = len(sp["dve_taps"])
                        if s == 0:
                            na, nd = BLOCK0.get(sp["g"], (na_full, nd_full))
                        else:
                            na, nd = na_full, nd_full
                        if na + nd == 0:
                            pair_ys.append(None)
                            continue
                        y = yp.tile([C, Y_ROWS * W], bf16, tag=f"y{sp['g']}")
                        kk = ks_base[oi]

                        def xview(dy, dx):
                            return xb3[:, PAD + r0 + dy: PAD + r0 + dy + Y_ROWS,
                                       PAD + dx: PAD + dx + W]

                        # products: DVE tensor_scalar (4x) + ACT activations
                        # (scale=k); each product folds into y via a DVE
                        # tensor_tensor add (2x) IMMEDIATELY so tmp tiles
                        # stay short-lived and both engines pipeline.  The
                        # first two ACT products of a large group pre-sum on
                        # gpsimd; the pair folds in at the end of the block.
                        a_list = sp["act_taps"][:na]
                        d_list = sp["dve_taps"][:nd]

                        def a_prod(t, dst):
                            dy, dx = sp["act_taps"][t]
                            nc.scalar.activation(dst, xview(dy, dx), IDENT,
                                                 bias=0.0,
                                                 scale=ks_sb[:, kk + t:kk + t + 1])

                        def d_prod(t, dst):
                            dy, dx = sp["dve_taps"][t]
                            c = kk + na_full + t
                            nc.vector.tensor_scalar_mul(dst, xview(dy, dx),
                                                        ks_sb[:, c:c + 1])

                        gp_tile = None
                        ai = di = 0
                        if GP_ADDS and len(a_list) >= 4:
                            ta = tmpp.tile([C, Y_ROWS * W], bf16, tag="pr")
                            tb = tmpp.tile([C, Y_ROWS * W], bf16, tag="pr")
                            a_prod(0, ta)
                            a_prod(1, tb)
                            nc.gpsimd.tensor_tensor(ta, ta, tb,
                                                    mybir.AluOpType.add)
                            gp_tile = ta
                            ai = 2
                        # init y with the first remaining product
                        if di < len(d_list):
                            d_prod(di, y)
                            di += 1
                        else:
                            a_prod(ai, y)
                            ai += 1
                        # round-robin the rest, folding immediately
                        while ai < len(a_list) or di < len(d_list):
                            take_a = ai < len(a_list) and (
                                di >= len(d_list)
                                or (ai - 2 if gp_tile is not None else ai) * len(d_list)
                                <= di * max(1, len(a_list) - (2 if gp_tile is not None else 0)))
                            tmp = tmpp.tile([C, Y_ROWS * W], bf16, tag="pr")
                            if take_a:
                                a_prod(ai, tmp)
                                ai += 1
                            else:
                                d_prod(di, tmp)
                                di += 1
                            nc.vector.tensor_tensor(y, y, tmp,
                                                    mybir.AluOpType.add)
                        if gp_tile is not None:
                            nc.vector.tensor_tensor(y, y, gp_tile,
                                                    mybir.AluOpType.add)
                        pair_ys.append(y.rearrange("p (r w) -> p r w", w=W))
                half = (s % 2) * SB_ROWS
                ys = [y3[:, half:half + SB_ROWS, :] if y3 is not None else None
                      for y3 in pair_ys]

                psum = psum_pool.tile([C, SB_ROWS * W], f32, tag="acc")
                for pi, (ta, tb) in enumerate(pairs):
                    for u in range(n_sub):
                        rhs = xq_pair_ap(PAD + r0 + u * SUB_ROWS, ta[0], tb[0])
                        nc.tensor.matmul(
                            psum[:, u * SUB_ROWS * W:(u + 1) * SUB_ROWS * W],
                            wt8v[:, pi], rhs,
                            start=(pi == 0), stop=False,
                            perf_mode=mybir.MatmulPerfMode.DoubleRow)
                if s < 2:
                    for pi0, (ta, tb) in enumerate(pairs0):
                        for u in range(n_sub):
                            rhs = xq_pair_ap(PAD + r0 + u * SUB_ROWS,
                                             ta[0], tb[0])
                            nc.tensor.matmul(
                                psum[:, u * SUB_ROWS * W:(u + 1) * SUB_ROWS * W],
                                wt8v[:, n_pair + pi0], rhs,
                                start=False, stop=False,
                                perf_mode=mybir.MatmulPerfMode.DoubleRow)
                if single is not None:
                    (di, dj), _ = single
                    for u in range(n_sub):
                        a0 = PAD + r0 + u * SUB_ROWS + di
                        rhs = xb3[:, a0: a0 + SUB_ROWS, PAD + dj: PAD + dj + W]
                        nc.tensor.matmul(
                            psum[:, u * SUB_ROWS * W:(u + 1) * SUB_ROWS * W],
                            wblk(SGL_BLK), rhs, start=False, stop=False)
                for oi in range(n_off):
                    if ys[oi] is None:
                        continue
                    for u in range(n_sub):
                        nc.tensor.matmul(
                            psum[:, u * SUB_ROWS * W:(u + 1) * SUB_ROWS * W],
                            wblk(PW_BLK + oi),
                            ys[oi][:, u * SUB_ROWS:(u + 1) * SUB_ROWS, :],
                            start=False, stop=False)

                # strip corrections: identity (unscaled) folds of corr_sb
                psum3 = psum.rearrange("p (r w) -> p r w", w=W)
                strip_mms = []
                for j, st in enumerate(strips):
                    if st["kind"] == "col":
                        dst = psum3[:, 0:SB_ROWS,
                                    st["fixed_out"]:st["fixed_out"] + 1]
                        src = corr_sb[:, j * H + r0: j * H + r0 + SB_ROWS]
                        strip_mms.append((dst, src))
                    elif r0 <= st["fixed_out"] < r0 + SB_ROWS:
                        lr = st["fixed_out"] - r0
                        strip_mms.append((psum3[:, lr:lr + 1, :],
                                          corr_sb[:, j * H: j * H + W]))
                for dst, src in strip_mms:
                    nc.tensor.matmul(dst, wblk(ID1_BLK), src,
                                     start=False, stop=False)

                # residual: scaled identity against bf16 x (last, stop=True)
                for u in range(n_sub):
                    a0 = PAD + r0 + u * SUB_ROWS
                    rhs = xb3[:, a0: a0 + SUB_ROWS, PAD: PAD + W]
                    nc.tensor.matmul(
                        psum[:, u * SUB_ROWS * W:(u + 1) * SUB_ROWS * W],
                        wblk(IDS_BLK), rhs, start=False, stop=(u == n_sub - 1))

                pending.append((psum, r0, s))
                # chase the PE tightly over the last super-blocks so the
                # evac+DMA tail after the final matmul is one SB deep
                limit = 2 if s < N_SB - 5 else (1 if s < N_SB - 1 else 0)
                while len(pending) > limit:
                    flush_evac()
            while pending:
                flush_evac()
    nc.finalize()
    return nc


def _make_in_maps(inputs):
    import ml_dtypes
    bfd = ml_dtypes.bfloat16
    f8d = ml_dtypes.float8_e4m3

    x = np.ascontiguousarray(inputs["x"], dtype=np.float32)
    (wt8, wc8, wtb, ks, beff, pairs, single, pairs0,
     off_specs, strips) = _build_weights(inputs)
    if "nc" not in _CACHE:
        _CACHE["nc"] = _build_program(pairs, single, pairs0, off_specs, strips)

    xpad_b = np.zeros((B, C, HP, WP), bfd)
    xpad_b[:, :, PAD:PAD + H, PAD:PAD + W] = x.astype(bfd)
    xpad_8 = np.zeros((B, C, HP, WP), f8d)
    xpad_8[:, :, PAD:PAD + H, PAD:PAD + W] = (x * (2.0 ** SX)).astype(f8d)
    # edge strips for the standalone correction pass: x's border lines
    # (src is 0 or W-1 / H-1), padded along the strip like xpad
    xe = np.zeros((B, C, 2 * HP + 2 * WP), f8d)
    xe[:, :, 0:HP] = xpad_8[:, :, :, PAD + 0]
    xe[:, :, HP:2 * HP] = xpad_8[:, :, :, PAD + W - 1]
    xe[:, :, 2 * HP:2 * HP + WP] = xpad_8[:, :, PAD + 0, :]
    xe[:, :, 2 * HP + WP:] = xpad_8[:, :, PAD + H - 1, :]
    beff_col = np.ascontiguousarray(beff.reshape(C, 1))
    ksc = np.ascontiguousarray(ks)
    return [
        {
            "xq": np.ascontiguousarray(xpad_8[b].reshape(C, HP * WP)),
            "xb": np.ascontiguousarray(xpad_b[b].reshape(C, HP * WP)),
            "xe": np.ascontiguousarray(xe[b]),
            "wt8": wt8,
            "wc8": wc8,
            "wtb": wtb,
            "ks": ksc,
            "beff": beff_col,
        }
        for b in range(B)
    ]


def kernel(**inputs):
    in_maps = _make_in_maps(inputs)
    from concourse.bass_utils import run_bass_kernel_spmd
    res = run_bass_kernel_spmd(_CACHE["nc"], in_maps, core_ids=list(range(N_CORES)))
    out = np.stack([res.results[b]["out"].reshape(C, H, W) for b in range(B)])
    return out.astype(np.float32)
